# revision 33
# baseline (speedup 1.0000x reference)
"""AttnReadout kernel for Trainium2, 8 NeuronCores, data-parallel over batch.

Math (per batch b, head i):
  c[i,e]    = bu[i,e] + sum_d Wv[i,e,d] * x[b, i, last_nodes[b,i], d]
  z[t,e]    = sum_d x[b,t,d] * Wu[i,e,d]          (t over O*N = 8192 tokens)
  s[t,e]    = sigmoid(z[t,e] + c[i,e])
  score[t]  = sum_e We[i,e] * s[t,e]
  alpha     = softmax(score)        (scores bounded by |We|_1, so exp without
                                     max-subtraction is safe; softmax is
                                     shift-invariant so results match)
  out[b,i]  = sum_t alpha[t] * x[b,t,:]

Trick: sigmoid(v) = (1 + tanh(v/2))/2 and the We-dot is linear, so
  score = sum_e (We_e/2)*tanh((z_e + c_e)/2) + sum_e We_e/2
Using tanh keeps every ACT function (tanh, exp, identity) in the single
`exp_and_others` table set -> no ACT table reloads. The /2 factors are
folded into the uploaded weights (exact in bf16), the +sum(We)/2 into the
exp bias.

Device dataflow per core (4 samples):
  - x in two DRAM layouts: fp8e4m3 transposed [d, t] for the projection
    (d=128 contraction averages fp8 noise; rel err 0.0019 vs 0.0016 bf16),
    bf16 natural-chunked [t, d+ones] for the weighted sum.
  - proj: PE matmul (Wu/2)^T stationary (bf16) x xT[d, 512] (fp8) -> PSUM
  - tanh: ACT over [128, 1024] with per-partition bias ch -> SBUF bf16
  - score: PE matmul with tanh tile [e,128] stationary, (We/2)[e,1] moving
    -> scores land token-on-partition in PSUM (2-group-deep software
    pipeline so the tanh has landed before the PE needs it; wedot and
    wsum emit BEFORE the proj whose zp-buffer WAR would stall the PE)
  - exp: ACT Exp(score + cw) -> alpha (bf16), no accum needed
  - wsum: PE matmul alpha[t,2] stationary x xn[t,129] moving; the ones
    column makes u[i,128] = Z_i, so softmax normalization costs nothing
  - out: DVE reciprocal of u[:,128] + per-partition scale -> DMA out
  wsum blocks (8 chunks) are spread one per two groups so the PE always
  has filler while a tanh is in flight. The last sample runs group-major
  with quarter-split exps so 3/4 of its weighted sum overlaps its own
  projection stream (short serial endgame).

All small weights ride in 2 packed DMAs (bf16 + f32) and the first xt
slice is issued before them: DMA descriptor issues serialize ~650ns apart
on the sequencer, so issue order dominates the ramp. HAM/p-state warmup
matmuls run on a memset dummy tile from t=0 (no DMA dependency).
Engine busy floor per core: PE ~75us (proj 16.4k + ldweights 4.1k +
wedot 16.5k + wsum 8.4k cycles/sample), ACT ~72us (64 tanh x ~1.1us);
both are within a few us of saturation mid-run. Thermal (HAM k=4)
throttling adds ~0-15us run-to-run variance.
"""

import numpy as np
import ml_dtypes

import concourse.bacc as bacc
import concourse.tile as tile
from concourse import mybir
from concourse.bass_utils import run_bass_kernel_spmd

# Note: fast weight load (walrus --enable-ldw-opt) was tried and rejected —
# walrus errors on bass's standalone InstLdweights form, so stationary loads
# run at 1 column/cycle here. fp8 DoubleRow matmuls measured SLOWER than
# bf16 on this silicon (probe: 82.7ns vs 57.7ns per wedot op) — only the
# mixed bf16-stationary x fp8-moving form is used (full speed, bit-exact
# vs host emulation).

BF = ml_dtypes.bfloat16
F8 = ml_dtypes.float8_e4m3
B, O, N, D = 32, 2, 4096, 128
NCORES = 8
BPC = B // NCORES          # samples per core
T = O * N                  # tokens per sample
CH = 512                   # projection chunk (free dim)
NG = T // (2 * CH)         # 8 tanh groups of 1024 per head
NC64 = T // 128            # 64 token chunks of 128

# packed bf16 weight blob column offsets: wu[2*128] wv[2*128] we[2] xl[8]
PK_WU = 0
PK_WV = 2 * D
PK_WE = 4 * D
PK_XL = 4 * D + O
PK_F = PK_XL + O * BPC


def _build_program():
    nc = bacc.Bacc("TRN2", target_bir_lowering=False)
    dt = mybir.dt
    f32, bf16, f8 = dt.float32, dt.bfloat16, dt.float8e4

    xt_d = nc.dram_tensor("xt", [BPC, D, T], f8, kind="ExternalInput")
    # xn carries a ones column per 128-token chunk (d index 128) so the wsum
    # matmul also produces Z = sum_t alpha[t] in u[:, 128] — no ACT accum_out
    xn_d = nc.dram_tensor("xn", [BPC, 2, D, 32 * (D + 1)], bf16, kind="ExternalInput")
    pkb_d = nc.dram_tensor("pkb", [D, PK_F], bf16, kind="ExternalInput")
    pkf_d = nc.dram_tensor("pkf", [D, 2 * O], f32, kind="ExternalInput")
    out_d = nc.dram_tensor("out", [BPC, O, D], f32, kind="ExternalOutput")

    Tanh = mybir.ActivationFunctionType.Tanh
    Exp = mybir.ActivationFunctionType.Exp
    Ident = mybir.ActivationFunctionType.Identity

    with tile.TileContext(nc) as tc:
        from contextlib import ExitStack

        with ExitStack() as ctx:
            singles = ctx.enter_context(tc.tile_pool(name="singles", bufs=1))
            xtp = ctx.enter_context(tc.tile_pool(name="xtp", bufs=3))
            xnp = ctx.enter_context(tc.tile_pool(name="xnp", bufs=3))
            zp = ctx.enter_context(tc.tile_pool(name="zp", bufs=2, space="PSUM"))
            sp = ctx.enter_context(tc.tile_pool(name="sp", bufs=6))
            # scores and the wsum accumulator u must own separate 2KB PSUM
            # banks: matmul start=True marks the whole zero-region pending,
            # which would clobber a co-resident accumulation
            scp = ctx.enter_context(tc.tile_pool(name="scp", bufs=3, space="PSUM"))
            up = ctx.enter_context(tc.tile_pool(name="up", bufs=1, space="PSUM"))
            smalls = ctx.enter_context(tc.tile_pool(name="smalls", bufs=2))

            # first xt slice for sample 0 goes out ahead of everything else
            xt0_sb = xtp.tile([D, T], f8, tag="xt")
            nc.sync.dma_start(out=xt0_sb[:, 0:512], in_=xt_d[0, :, 0:512])

            pkb_sb = singles.tile([D, PK_F], bf16)
            nc.sync.dma_start(out=pkb_sb, in_=pkb_d[:])
            pkf_sb = singles.tile([D, 2 * O], f32)
            nc.sync.dma_start(out=pkf_sb, in_=pkf_d[:])
            wu_sb = pkb_sb[:, PK_WU : PK_WU + 2 * D].rearrange(
                "p (i e) -> p i e", i=O
            )
            wv_sb = pkb_sb[:, PK_WV : PK_WV + 2 * D].rearrange(
                "p (i e) -> p i e", i=O
            )
            we_sb = pkb_sb[:, PK_WE : PK_WE + O]
            xl_sb = pkb_sb[:, PK_XL : PK_XL + O * BPC]
            bu_sb = pkf_sb[:, 0:O]
            cw_sb = pkf_sb[:, O : 2 * O]

            # HAM warmup: dense dummy matmuls on a never-written (garbage)
            # SBUF tile keep the PE spinning from t=0 — no DMA dependency —
            # so the real stream starts at the un-throttled 2.4 GHz clock.
            dummy_sb = singles.tile([D, D], bf16)
            nc.gpsimd.memset(dummy_sb, 0.0)
            warm_ps = scp.tile([D, D], f32, tag="scu")
            for w in range(56):
                nc.tensor.matmul(
                    warm_ps[:, 0:D], dummy_sb, dummy_sb, start=True, stop=True
                )

            # per-(sample, head) tanh bias ch[e, j] = (xv + bu)/2, j = i*BPC + b
            # (wv and bu are uploaded pre-halved)
            # rides the scores pool rotation: single start=True write, fully
            # consumed (via ch_sb) before the first wedot touches the pool
            c_ps = scp.tile([D, O * BPC], f32, tag="scu")
            for i in range(O):
                nc.tensor.matmul(
                    c_ps[:, i * BPC : (i + 1) * BPC],
                    wv_sb[:, i, :],
                    xl_sb[:, i * BPC : (i + 1) * BPC],
                    start=True,
                    stop=True,
                )
            ch_sb = singles.tile([D, O * BPC], f32)
            for i in range(O):
                nc.scalar.activation(
                    out=ch_sb[:, i * BPC : (i + 1) * BPC],
                    in_=c_ps[:, i * BPC : (i + 1) * BPC],
                    func=Ident,
                    bias=bu_sb[:, i : i + 1],
                )

            # Software-pipelined emission, 2 groups deep: each group's score
            # reduction is emitted two projections later so the ACT tanh has
            # always landed by the time the PE reaches it.
            samples = {}

            def start_sample(b, xt_sb=None):
                if xt_sb is None:
                    xt_sb = xtp.tile([D, T], f8, tag="xt")
                    first = [0]
                else:
                    first = [512]  # slice [0:512] already in flight
                bounds = (
                    first + [1024, 2048, 3072, 4096, 6144, T]
                    if b == 0
                    else [q * (T // 4) for q in range(4)] + [T]
                )
                for lo, hi in zip(bounds[:-1], bounds[1:]):
                    if lo >= hi:
                        continue
                    nc.sync.dma_start(out=xt_sb[:, lo:hi], in_=xt_d[b, :, lo:hi])
                xn_sb = xnp.tile([D, NC64, D + 1], bf16, tag="xn")
                nc.sync.dma_start(
                    out=xn_sb.rearrange("p (g c) d -> p g c d", g=2),
                    in_=xn_d[b].rearrange("g p (c d) -> p g c d", c=32),
                )
                scu = scp.tile([D, D], f32, tag="scu")
                scores = scu.rearrange("p (i c) -> p i c", i=O)
                u_ap = up.tile([O, D + 1], f32, tag="u")
                # alpha chunk-major [d, c, i]: contiguous [t, 2] wsum slices
                alpha_sb = smalls.tile([D, NC64, O], bf16, tag="alpha")
                samples[b] = [xt_sb, xn_sb, scores, u_ap, alpha_sb]

            def emit_exp(b, i, half):
                # half: (lo, hi) chunk range covered by this exp, or full
                scores = samples[b][2]
                alpha_sb = samples[b][4]
                lo, hi = half
                nc.scalar.activation(
                    out=alpha_sb[:, lo:hi, i],
                    in_=scores[:, i, lo:hi],
                    func=Exp,
                    bias=cw_sb[:, i : i + 1],
                )

            def emit_wedot(b, i, g, t_flat):
                scores = samples[b][2]
                last = b == BPC - 1
                for sub in range(2 * CH // D):
                    col = g * (2 * CH // D) + sub
                    nc.tensor.matmul(
                        scores[:, i, col : col + 1],
                        t_flat[:, sub * D : (sub + 1) * D],
                        we_sb[:, i : i + 1],
                        start=True,
                        stop=True,
                    )
                if last and g == NG // 2 - 1:
                    # early partial exps so most of the weighted sum overlaps
                    # the remaining projection groups of the final sample
                    emit_exp(b, i, (0, NC64 // 2))
                    if i == O - 1:
                        for k in range(4):
                            deferred_wsum.append((b, k * 8, 8))
                if last and g == NG - 3:
                    emit_exp(b, i, (NC64 // 2, 3 * NC64 // 4))
                    if i == O - 1:
                        for k in range(2):
                            deferred_wsum.append((b, NC64 // 2 + k * 8, 8))
                if g == NG - 1:
                    if last:
                        emit_exp(b, i, (3 * NC64 // 4, NC64))
                    else:
                        emit_exp(b, i, (0, NC64))
                    if i == O - 1:
                        emit_tail(b)

            WSB = 8  # wsum block size; blocks interleave with the next
            deferred_wsum = []  # (b, start_chunk, n_chunks) queue

            def emit_tail(b):
                lo = 3 * NC64 // 4 if b == BPC - 1 else 0
                for blk in range((NC64 - lo) // WSB):
                    deferred_wsum.append((b, lo + blk * WSB, WSB))

            def emit_wsum_block():
                b, start, ln = deferred_wsum.pop(0)
                xn_sb, u_ap, alpha_sb = samples[b][1], samples[b][3], samples[b][4]
                # u[i, 0:128] = sum_t alpha[t, i] * x[t, d]; u[i, 128] = Z_i
                for c in range(start, start + ln):
                    nc.tensor.matmul(
                        u_ap,
                        alpha_sb[:, c, :],
                        xn_sb[:, c, :],
                        start=(c == 0),
                        stop=(c == NC64 - 1),
                    )
                if start + ln == NC64:
                    zinv_sb = smalls.tile([O, 1], f32, tag="zinv")
                    nc.vector.reciprocal(out=zinv_sb, in_=u_ap[:, D : D + 1])
                    o_sb = smalls.tile([O, D], f32, tag="osb")
                    nc.vector.tensor_scalar_mul(o_sb, u_ap[:, 0:D], zinv_sb)
                    nc.sync.dma_start(out=out_d[b], in_=o_sb)

            pending = []
            it_count = [0]

            def emit_group(b, i, g):
                # wedot/wsum first: they stand between the previous tanh and
                # the proj that waits on the zp buffer that tanh still reads
                if len(pending) >= 2:
                    emit_wedot(*pending.pop(0))
                it_count[0] += 1
                if deferred_wsum and (it_count[0] % 2 == 0 or b == BPC - 1):
                    emit_wsum_block()
                xt_sb = samples[b][0]
                z_ps = zp.tile([D, 2, CH], f32)
                for h in range(2):
                    c = g * 2 + h
                    nc.tensor.matmul(
                        z_ps[:, h, :],
                        wu_sb[:, i, :],
                        xt_sb[:, c * CH : (c + 1) * CH],
                        start=True,
                        stop=True,
                    )
                t_sb = sp.tile([D, 2, CH], bf16)
                j = i * BPC + b
                nc.scalar.activation(
                    out=t_sb.rearrange("p a b -> p (a b)"),
                    in_=z_ps.rearrange("p a b -> p (a b)"),
                    func=Tanh,
                    bias=ch_sb[:, j : j + 1],
                )
                pending.append((b, i, g, t_sb.rearrange("p a b -> p (a b)")))

            start_sample(0, xt0_sb)
            for b in range(BPC):
                if b > 0:
                    start_sample(b)
                if b == BPC - 1:
                    # group-major on the last sample: both heads' first-half
                    # scores complete mid-sample, unlocking the early exp
                    for g in range(NG):
                        for i in range(O):
                            emit_group(b, i, g)
                else:
                    for i in range(O):
                        for g in range(NG):
                            emit_group(b, i, g)
            while pending:
                emit_wedot(*pending.pop(0))
            while deferred_wsum:
                emit_wsum_block()

    nc.compile()
    return nc


def _prep_core_inputs(x, Wu, bu, Wv, We, last_nodes):
    """Host-side input marshalling: dtype cast + layout (weights pre-halved
    for the tanh formulation). Returns per-core input maps."""
    x = np.ascontiguousarray(x, dtype=np.float32)
    ln = np.asarray(last_nodes).astype(np.int64)
    xb = x.reshape(B, T, D)
    xbf = xb.astype(BF)                                  # [B, T, D] bf16
    # proj moving side in fp8e4m3: d=128 contraction averages the ~3.6%
    # quantization noise; measured end-to-end rel err 0.0019 vs 0.0016 bf16
    xt = np.ascontiguousarray(xb.transpose(0, 2, 1).astype(F8))  # [B, D, T]
    # natural-chunked layout with a trailing ones column per chunk:
    # xn[b, g, p, cc*(D+1) + d] = xb[b, (g*32 + cc)*128 + p, d], d<128; 1.0 at d=128
    xn5 = np.concatenate(
        [xbf.reshape(B, 2, 32, D, D), np.ones((B, 2, 32, D, 1), BF)], axis=-1
    )
    xn = np.ascontiguousarray(
        xn5.transpose(0, 1, 3, 2, 4).reshape(B, 2, D, 32 * (D + 1))
    )
    # x_last gather, transposed: xlT[core][d, j], j = i*BPC + b_local
    xl = xb[np.arange(B)[:, None], ln + np.arange(O)[None, :] * N]   # [B, O, D] f32
    # wuT[d, i, e] = Wu[i, e, d] / 2  (tanh halving, exact in bf16)
    wuT = (Wu * 0.5).transpose(2, 0, 1).astype(BF).reshape(D, 2 * D)
    wvT = (Wv * 0.5).transpose(2, 0, 1).astype(BF).reshape(D, 2 * D)
    we2 = (We * 0.5).astype(BF).T                        # [e, i] = We[i, e]/2
    bu2 = np.ascontiguousarray((bu * 0.5).astype(np.float32).T)  # [e, i]
    # exp bias: cw[i] = sum_e We[i, e]/2, replicated on all partitions
    cw = np.float32(0.5) * We.astype(np.float32).sum(axis=1)     # [O]
    cw2 = np.broadcast_to(cw[None, :], (D, O)).astype(np.float32)
    pkf = np.ascontiguousarray(np.concatenate([bu2, cw2], axis=1))  # [D, 4] f32

    maps = []
    for core in range(NCORES):
        sl = slice(core * BPC, (core + 1) * BPC)
        xlc = xl[sl]                                     # [BPC, O, D]
        xlT = xlc.transpose(2, 1, 0).reshape(D, O * BPC).astype(BF)  # [d, i*BPC+b]
        pkb = np.ascontiguousarray(np.concatenate([wuT, wvT, we2, xlT], axis=1))
        assert pkb.shape == (D, PK_F)
        maps.append(
            {
                "xt": xt[sl],
                "xn": xn[sl],
                "pkb": pkb,
                "pkf": pkf,
            }
        )
    return maps


_CACHE = {}
TRACE = False


def kernel(**inputs):
    x = np.asarray(inputs["x"])
    Wu = np.asarray(inputs["Wu"], dtype=np.float32)
    bu = np.asarray(inputs["bu"], dtype=np.float32)
    Wv = np.asarray(inputs["Wv"], dtype=np.float32)
    We = np.asarray(inputs["We"], dtype=np.float32)
    last_nodes = np.asarray(inputs["last_nodes"])

    maps = _prep_core_inputs(x, Wu, bu, Wv, We, last_nodes)
    if "nc" not in _CACHE:
        _CACHE["nc"] = _build_program()
    nc = _CACHE["nc"]
    res = run_bass_kernel_spmd(nc, maps, list(range(NCORES)), trace=TRACE)
    _CACHE["last_res"] = res
    outs = [np.asarray(r["out"], dtype=np.float32) for r in res.results]
    return np.concatenate(outs, axis=0)  # [B, O, D]


if __name__ == "__main__":
    rng = np.random.default_rng(0)
    x = rng.standard_normal((B, O, N, D), dtype=np.float32)
    Wu = rng.standard_normal((O, D, D), dtype=np.float32) * 0.09
    bu = np.zeros((O, D), np.float32)
    Wv = rng.standard_normal((O, D, D), dtype=np.float32) * 0.09
    We = rng.standard_normal((O, D), dtype=np.float32) * 0.09
    ln = rng.integers(0, N, size=(B, O)).astype(np.int64)
    out = kernel(x=x, Wu=Wu, bu=bu, Wv=Wv, We=We, last_nodes=ln)
    print(out.shape, out.dtype)


# revision 34
# speedup vs baseline: 1.0083x; 1.0083x over previous
"""AttnReadout kernel for Trainium2, 8 NeuronCores, data-parallel over batch.

Math (per batch b, head i):
  c[i,e]    = bu[i,e] + sum_d Wv[i,e,d] * x[b, i, last_nodes[b,i], d]
  z[t,e]    = sum_d x[b,t,d] * Wu[i,e,d]          (t over O*N = 8192 tokens)
  s[t,e]    = sigmoid(z[t,e] + c[i,e])
  score[t]  = sum_e We[i,e] * s[t,e]
  alpha     = softmax(score)        (scores bounded by |We|_1, so exp without
                                     max-subtraction is safe; softmax is
                                     shift-invariant so results match)
  out[b,i]  = sum_t alpha[t] * x[b,t,:]

Trick: sigmoid(v) = (1 + tanh(v/2))/2 and the We-dot is linear, so
  score = sum_e (We_e/2)*tanh((z_e + c_e)/2) + sum_e We_e/2
Using tanh keeps every ACT function (tanh, exp, identity) in the single
`exp_and_others` table set -> no ACT table reloads. The /2 factors are
folded into the uploaded weights (exact in bf16), the +sum(We)/2 into the
exp bias.

Device dataflow per core (4 samples):
  - x in two DRAM layouts: fp8e4m3 transposed [d, t] for the projection
    (d=128 contraction averages fp8 noise; rel err 0.0019 vs 0.0016 bf16),
    bf16 natural-chunked [t, d+ones] for the weighted sum.
  - proj: PE matmul (Wu/2)^T stationary (bf16) x xT[d, 512] (fp8) -> PSUM
  - tanh: ACT over [128, 1024] with per-partition bias ch -> SBUF bf16
  - score: PE matmul with tanh tile [e,128] stationary, (We/2)[e,1] moving
    -> scores land token-on-partition in PSUM (2-group-deep software
    pipeline so the tanh has landed before the PE needs it; wedot and
    wsum emit BEFORE the proj whose zp-buffer WAR would stall the PE)
  - exp: ACT Exp(score + cw) -> alpha (bf16), no accum needed
  - wsum: PE matmul alpha[t,2] stationary x xn[t,129] moving; the ones
    column makes u[i,128] = Z_i, so softmax normalization costs nothing
  - out: DVE reciprocal of u[:,128] + per-partition scale -> DMA out
  wsum blocks (8 chunks) are spread one per two groups so the PE always
  has filler while a tanh is in flight. The last sample runs group-major
  with quarter-split exps so 3/4 of its weighted sum overlaps its own
  projection stream (short serial endgame).

All small weights ride in 2 packed DMAs (bf16 + f32) and the first xt
slice is issued before them: DMA descriptor issues serialize ~650ns apart
on the sequencer, so issue order dominates the ramp. HAM/p-state warmup
matmuls run on a memset dummy tile from t=0 (no DMA dependency).
Engine busy floor per core: PE ~75us (proj 16.4k + ldweights 4.1k +
wedot 16.5k + wsum 8.4k cycles/sample), ACT ~72us (64 tanh x ~1.1us);
both are within a few us of saturation mid-run. Thermal (HAM k=4)
throttling adds ~0-15us run-to-run variance.
"""

import numpy as np
import ml_dtypes

import concourse.bacc as bacc
import concourse.tile as tile
from concourse import mybir
from concourse.bass_utils import run_bass_kernel_spmd

# Note: fast weight load (walrus --enable-ldw-opt) was tried and rejected —
# walrus errors on bass's standalone InstLdweights form, so stationary loads
# run at 1 column/cycle here. fp8 DoubleRow matmuls measured SLOWER than
# bf16 on this silicon (probe: 82.7ns vs 57.7ns per wedot op) — only the
# mixed bf16-stationary x fp8-moving form is used (full speed, bit-exact
# vs host emulation).

BF = ml_dtypes.bfloat16
F8 = ml_dtypes.float8_e4m3
B, O, N, D = 32, 2, 4096, 128
NCORES = 8
BPC = B // NCORES          # samples per core
T = O * N                  # tokens per sample
CH = 512                   # projection chunk (free dim)
NG = T // (2 * CH)         # 8 tanh groups of 1024 per head
NC64 = T // 128            # 64 token chunks of 128

# packed bf16 weight blob column offsets: wu[2*128] wv[2*128] we[2] xl[8]
PK_WU = 0
PK_WV = 2 * D
PK_WE = 4 * D
PK_XL = 4 * D + O
PK_F = PK_XL + O * BPC


def _build_program():
    nc = bacc.Bacc("TRN2", target_bir_lowering=False)
    dt = mybir.dt
    f32, bf16, f8 = dt.float32, dt.bfloat16, dt.float8e4

    xt_d = nc.dram_tensor("xt", [BPC, D, T], f8, kind="ExternalInput")
    # xn carries a ones column per 128-token chunk (d index 128) so the wsum
    # matmul also produces Z = sum_t alpha[t] in u[:, 128] — no ACT accum_out
    xn_d = nc.dram_tensor("xn", [BPC, 2, D, 32 * (D + 1)], bf16, kind="ExternalInput")
    pkb_d = nc.dram_tensor("pkb", [D, PK_F], bf16, kind="ExternalInput")
    pkf_d = nc.dram_tensor("pkf", [D, 2 * O], f32, kind="ExternalInput")
    out_d = nc.dram_tensor("out", [BPC, O, D], f32, kind="ExternalOutput")

    Tanh = mybir.ActivationFunctionType.Tanh
    Exp = mybir.ActivationFunctionType.Exp
    Ident = mybir.ActivationFunctionType.Identity

    with tile.TileContext(nc) as tc:
        from contextlib import ExitStack

        with ExitStack() as ctx:
            singles = ctx.enter_context(tc.tile_pool(name="singles", bufs=1))
            xtp = ctx.enter_context(tc.tile_pool(name="xtp", bufs=3))
            xnp = ctx.enter_context(tc.tile_pool(name="xnp", bufs=3))
            zp = ctx.enter_context(tc.tile_pool(name="zp", bufs=2, space="PSUM"))
            sp = ctx.enter_context(tc.tile_pool(name="sp", bufs=6))
            # scores and the wsum accumulator u must own separate 2KB PSUM
            # banks: matmul start=True marks the whole zero-region pending,
            # which would clobber a co-resident accumulation
            scp = ctx.enter_context(tc.tile_pool(name="scp", bufs=3, space="PSUM"))
            up = ctx.enter_context(tc.tile_pool(name="up", bufs=1, space="PSUM"))
            smalls = ctx.enter_context(tc.tile_pool(name="smalls", bufs=2))

            # first xt slice for sample 0 goes out ahead of everything else
            xt0_sb = xtp.tile([D, T], f8, tag="xt")
            nc.sync.dma_start(out=xt0_sb[:, 0:512], in_=xt_d[0, :, 0:512])

            pkb_sb = singles.tile([D, PK_F], bf16)
            nc.sync.dma_start(out=pkb_sb, in_=pkb_d[:])
            pkf_sb = singles.tile([D, 2 * O], f32)
            nc.sync.dma_start(out=pkf_sb, in_=pkf_d[:])
            wu_sb = pkb_sb[:, PK_WU : PK_WU + 2 * D].rearrange(
                "p (i e) -> p i e", i=O
            )
            wv_sb = pkb_sb[:, PK_WV : PK_WV + 2 * D].rearrange(
                "p (i e) -> p i e", i=O
            )
            we_sb = pkb_sb[:, PK_WE : PK_WE + O]
            xl_sb = pkb_sb[:, PK_XL : PK_XL + O * BPC]
            bu_sb = pkf_sb[:, 0:O]
            cw_sb = pkf_sb[:, O : 2 * O]

            # HAM warmup: dense dummy matmuls on a never-written (garbage)
            # SBUF tile keep the PE spinning from t=0 — no DMA dependency —
            # so the real stream starts at the un-throttled 2.4 GHz clock.
            dummy_sb = singles.tile([D, D], bf16)
            nc.gpsimd.memset(dummy_sb, 0.0)
            warm_ps = scp.tile([D, D], f32, tag="scu")
            for w in range(56):
                nc.tensor.matmul(
                    warm_ps[:, 0:D], dummy_sb, dummy_sb, start=True, stop=True
                )

            # per-(sample, head) tanh bias ch[e, j] = (xv + bu)/2, j = i*BPC + b
            # (wv and bu are uploaded pre-halved)
            # rides the scores pool rotation: single start=True write, fully
            # consumed (via ch_sb) before the first wedot touches the pool
            c_ps = scp.tile([D, O * BPC], f32, tag="scu")
            for i in range(O):
                nc.tensor.matmul(
                    c_ps[:, i * BPC : (i + 1) * BPC],
                    wv_sb[:, i, :],
                    xl_sb[:, i * BPC : (i + 1) * BPC],
                    start=True,
                    stop=True,
                )
            ch_sb = singles.tile([D, O * BPC], f32)
            for i in range(O):
                nc.scalar.activation(
                    out=ch_sb[:, i * BPC : (i + 1) * BPC],
                    in_=c_ps[:, i * BPC : (i + 1) * BPC],
                    func=Ident,
                    bias=bu_sb[:, i : i + 1],
                )

            # Software-pipelined emission, 2 groups deep: each group's score
            # reduction is emitted two projections later so the ACT tanh has
            # always landed by the time the PE reaches it.
            samples = {}

            def start_sample(b, xt_sb=None):
                if xt_sb is None:
                    xt_sb = xtp.tile([D, T], f8, tag="xt")
                    first = [0]
                else:
                    first = [512]  # slice [0:512] already in flight
                bounds = (
                    first + [1024, 2048, 3072, 4096, 6144, T]
                    if b == 0
                    else [q * (T // 4) for q in range(4)] + [T]
                )
                for lo, hi in zip(bounds[:-1], bounds[1:]):
                    if lo >= hi:
                        continue
                    nc.sync.dma_start(out=xt_sb[:, lo:hi], in_=xt_d[b, :, lo:hi])
                xn_sb = xnp.tile([D, NC64, D + 1], bf16, tag="xn")
                nc.sync.dma_start(
                    out=xn_sb.rearrange("p (g c) d -> p g c d", g=2),
                    in_=xn_d[b].rearrange("g p (c d) -> p g c d", c=32),
                )
                scu = scp.tile([D, D], f32, tag="scu")
                scores = scu.rearrange("p (i c) -> p i c", i=O)
                u_ap = up.tile([O, D + 1], f32, tag="u")
                # alpha chunk-major [d, c, i]: contiguous [t, 2] wsum slices
                alpha_sb = smalls.tile([D, NC64, O], bf16, tag="alpha")
                samples[b] = [xt_sb, xn_sb, scores, u_ap, alpha_sb]

            def emit_exp(b, i, half):
                # half: (lo, hi) chunk range covered by this exp, or full
                scores = samples[b][2]
                alpha_sb = samples[b][4]
                lo, hi = half
                nc.scalar.activation(
                    out=alpha_sb[:, lo:hi, i],
                    in_=scores[:, i, lo:hi],
                    func=Exp,
                    bias=cw_sb[:, i : i + 1],
                )

            def emit_wedot(b, i, g, t_flat):
                scores = samples[b][2]
                last = b == BPC - 1
                for sub in range(2 * CH // D):
                    col = g * (2 * CH // D) + sub
                    nc.tensor.matmul(
                        scores[:, i, col : col + 1],
                        t_flat[:, sub * D : (sub + 1) * D],
                        we_sb[:, i : i + 1],
                        start=True,
                        stop=True,
                    )
                if last and g == NG // 2 - 1:
                    # early partial exps so most of the weighted sum overlaps
                    # the remaining projection groups of the final sample
                    emit_exp(b, i, (0, NC64 // 2))
                    if i == O - 1:
                        for k in range(4):
                            deferred_wsum.append((b, k * 8, 8))
                if last and g == NG - 3:
                    emit_exp(b, i, (NC64 // 2, 3 * NC64 // 4))
                    if i == O - 1:
                        for k in range(2):
                            deferred_wsum.append((b, NC64 // 2 + k * 8, 8))
                if g == NG - 1:
                    if last:
                        emit_exp(b, i, (3 * NC64 // 4, NC64))
                    else:
                        emit_exp(b, i, (0, NC64))
                    if i == O - 1:
                        emit_tail(b)

            WSB = 8  # wsum block size; blocks interleave with the next
            deferred_wsum = []  # (b, start_chunk, n_chunks) queue

            def emit_tail(b):
                lo = 3 * NC64 // 4 if b == BPC - 1 else 0
                for blk in range((NC64 - lo) // WSB):
                    deferred_wsum.append((b, lo + blk * WSB, WSB))

            def emit_wsum_block():
                b, start, ln = deferred_wsum.pop(0)
                xn_sb, u_ap, alpha_sb = samples[b][1], samples[b][3], samples[b][4]
                # u[i, 0:128] = sum_t alpha[t, i] * x[t, d]; u[i, 128] = Z_i
                for c in range(start, start + ln):
                    nc.tensor.matmul(
                        u_ap,
                        alpha_sb[:, c, :],
                        xn_sb[:, c, :],
                        start=(c == 0),
                        stop=(c == NC64 - 1),
                    )
                if start + ln == NC64:
                    zinv_sb = smalls.tile([O, 1], f32, tag="zinv")
                    nc.vector.reciprocal(out=zinv_sb, in_=u_ap[:, D : D + 1])
                    o_sb = smalls.tile([O, D], f32, tag="osb")
                    nc.vector.tensor_scalar_mul(o_sb, u_ap[:, 0:D], zinv_sb)
                    nc.sync.dma_start(out=out_d[b], in_=o_sb)

            pending = []
            it_count = [0]

            def emit_group(b, i, g):
                # wedot/wsum first: they stand between the previous tanh and
                # the proj that waits on the zp buffer that tanh still reads
                if len(pending) >= 2:
                    emit_wedot(*pending.pop(0))
                it_count[0] += 1
                if deferred_wsum and (it_count[0] % 2 == 0 or b == BPC - 1):
                    emit_wsum_block()
                xt_sb = samples[b][0]
                z_ps = zp.tile([D, 2, CH], f32)
                for h in range(2):
                    c = g * 2 + h
                    nc.tensor.matmul(
                        z_ps[:, h, :],
                        wu_sb[:, i, :],
                        xt_sb[:, c * CH : (c + 1) * CH],
                        start=True,
                        stop=True,
                    )
                t_sb = sp.tile([D, 2, CH], bf16)
                j = i * BPC + b
                nc.scalar.activation(
                    out=t_sb.rearrange("p a b -> p (a b)"),
                    in_=z_ps.rearrange("p a b -> p (a b)"),
                    func=Tanh,
                    bias=ch_sb[:, j : j + 1],
                )
                pending.append((b, i, g, t_sb.rearrange("p a b -> p (a b)")))

            start_sample(0, xt0_sb)
            for b in range(BPC):
                if b == BPC - 1:
                    # group-major on the last sample: both heads' first-half
                    # scores complete mid-sample, unlocking the early exp
                    order = [(i, g) for g in range(NG) for i in range(O)]
                else:
                    order = [(i, g) for i in range(O) for g in range(NG)]
                for k, (i, g) in enumerate(order):
                    emit_group(b, i, g)
                    # prefetch the next sample's DMAs 4 groups early so its
                    # first projection never waits on the xt transfer
                    if k == 2 * NG - 5 and b + 1 < BPC:
                        start_sample(b + 1)
            while pending:
                emit_wedot(*pending.pop(0))
            while deferred_wsum:
                emit_wsum_block()

    nc.compile()
    return nc


def _prep_core_inputs(x, Wu, bu, Wv, We, last_nodes):
    """Host-side input marshalling: dtype cast + layout (weights pre-halved
    for the tanh formulation). Returns per-core input maps."""
    x = np.ascontiguousarray(x, dtype=np.float32)
    ln = np.asarray(last_nodes).astype(np.int64)
    xb = x.reshape(B, T, D)
    xbf = xb.astype(BF)                                  # [B, T, D] bf16
    # proj moving side in fp8e4m3: d=128 contraction averages the ~3.6%
    # quantization noise; measured end-to-end rel err 0.0019 vs 0.0016 bf16
    xt = np.ascontiguousarray(xb.transpose(0, 2, 1).astype(F8))  # [B, D, T]
    # natural-chunked layout with a trailing ones column per chunk:
    # xn[b, g, p, cc*(D+1) + d] = xb[b, (g*32 + cc)*128 + p, d], d<128; 1.0 at d=128
    xn5 = np.concatenate(
        [xbf.reshape(B, 2, 32, D, D), np.ones((B, 2, 32, D, 1), BF)], axis=-1
    )
    xn = np.ascontiguousarray(
        xn5.transpose(0, 1, 3, 2, 4).reshape(B, 2, D, 32 * (D + 1))
    )
    # x_last gather, transposed: xlT[core][d, j], j = i*BPC + b_local
    xl = xb[np.arange(B)[:, None], ln + np.arange(O)[None, :] * N]   # [B, O, D] f32
    # wuT[d, i, e] = Wu[i, e, d] / 2  (tanh halving, exact in bf16)
    wuT = (Wu * 0.5).transpose(2, 0, 1).astype(BF).reshape(D, 2 * D)
    wvT = (Wv * 0.5).transpose(2, 0, 1).astype(BF).reshape(D, 2 * D)
    we2 = (We * 0.5).astype(BF).T                        # [e, i] = We[i, e]/2
    bu2 = np.ascontiguousarray((bu * 0.5).astype(np.float32).T)  # [e, i]
    # exp bias: cw[i] = sum_e We[i, e]/2, replicated on all partitions
    cw = np.float32(0.5) * We.astype(np.float32).sum(axis=1)     # [O]
    cw2 = np.broadcast_to(cw[None, :], (D, O)).astype(np.float32)
    pkf = np.ascontiguousarray(np.concatenate([bu2, cw2], axis=1))  # [D, 4] f32

    maps = []
    for core in range(NCORES):
        sl = slice(core * BPC, (core + 1) * BPC)
        xlc = xl[sl]                                     # [BPC, O, D]
        xlT = xlc.transpose(2, 1, 0).reshape(D, O * BPC).astype(BF)  # [d, i*BPC+b]
        pkb = np.ascontiguousarray(np.concatenate([wuT, wvT, we2, xlT], axis=1))
        assert pkb.shape == (D, PK_F)
        maps.append(
            {
                "xt": xt[sl],
                "xn": xn[sl],
                "pkb": pkb,
                "pkf": pkf,
            }
        )
    return maps


_CACHE = {}
TRACE = False


def kernel(**inputs):
    x = np.asarray(inputs["x"])
    Wu = np.asarray(inputs["Wu"], dtype=np.float32)
    bu = np.asarray(inputs["bu"], dtype=np.float32)
    Wv = np.asarray(inputs["Wv"], dtype=np.float32)
    We = np.asarray(inputs["We"], dtype=np.float32)
    last_nodes = np.asarray(inputs["last_nodes"])

    maps = _prep_core_inputs(x, Wu, bu, Wv, We, last_nodes)
    if "nc" not in _CACHE:
        _CACHE["nc"] = _build_program()
    nc = _CACHE["nc"]
    res = run_bass_kernel_spmd(nc, maps, list(range(NCORES)), trace=TRACE)
    _CACHE["last_res"] = res
    outs = [np.asarray(r["out"], dtype=np.float32) for r in res.results]
    return np.concatenate(outs, axis=0)  # [B, O, D]


if __name__ == "__main__":
    rng = np.random.default_rng(0)
    x = rng.standard_normal((B, O, N, D), dtype=np.float32)
    Wu = rng.standard_normal((O, D, D), dtype=np.float32) * 0.09
    bu = np.zeros((O, D), np.float32)
    Wv = rng.standard_normal((O, D, D), dtype=np.float32) * 0.09
    We = rng.standard_normal((O, D), dtype=np.float32) * 0.09
    ln = rng.integers(0, N, size=(B, O)).astype(np.int64)
    out = kernel(x=x, Wu=Wu, bu=bu, Wv=Wv, We=We, last_nodes=ln)
    print(out.shape, out.dtype)
